# revision 80
# baseline (speedup 1.0000x reference)
"""MultiHeadAttention + RoPE kernel for 8 Trainium2 NeuronCores.

Sharding: core c in 0..7 -> batch b = c//4, head-group hg = c%4 (4 heads
each).  Each core computes its 4 heads' attention for its batch and a
partial output projection y_part = out_heads @ wo[head rows]; the host
sums the 4 partials per batch (fp32) and adds bo.

v2 dataflow (all matmuls bf16, 1 cyc/row; exp on ACT is the bottleneck
engine so everything else is pipelined around a continuous exp stream):
  - host converts inputs to bf16; xT = x[b].T so contraction D is on
    partitions; DMA ~9 MB/core instead of 20.
  - Q/K projected in 512-col chunks; bias via tensor_scalar; RoPE via a
    signed-permutation matmul (rot) + cos/sin combines (DVE), final add
    on GPSIMD; outputs bf16.
  - V natural [S, depth] + ones column -> V' [128, sk, h, 65] bf16.
  - scores ST [128 keys, q] = matmul(lhsT=KT tile, rhs=QT); exp on ACT
    with scale=1/8 and bias=-4 folded in (shift cancels in normalize);
    PV accumulates out'T [65, q]; row 64 = softmax denominator.
  - normalize: DVE recip + gpsimd partition-broadcast, write outT bf16.
  - O-proj per 128-row tile, K=64-accumulated over the 4 local heads;
    y evacuated to bf16 and DMA'd; host sums partials in fp32.
  - emission order interleaves phases so the first exp lands ~12 us in
    and ACT stays busy: K(kb0) -> Q(qb0) -> V(0..3) -> phase B(qb0)
    with V(4..15) and Q(qb1) emitted inside, phase C(qb0) inside
    phase B(qb1), phase C(qb1) tail.
"""

import numpy as np
import ml_dtypes

import concourse.bacc as bacc
import concourse.mybir as mybir
from concourse.tile import TileContext

try:  # persistent XLA compile cache: repeat processes skip the ~4min compile
    import jax as _jax
    _jax.config.update("jax_compilation_cache_dir", "/tmp/jax_comp_cache")
    _jax.config.update("jax_persistent_cache_min_compile_time_secs", 1.0)
except Exception:
    pass

B, S, DM, H, DH = 2, 2048, 1024, 16, 64
NCORES = 8
HL = 4                 # heads per core
DHL = HL * DH          # 256
KCH = DM // 128        # 8 k-chunks of the model-dim contraction
SKT = S // 128         # 16 key tiles
QT_TILES = DHL // 128  # 2 m-tiles for the Q/K projections
QB = 1024              # phase-B q block
NQB = S // QB          # 2
PC = 1024              # projection chunk width

F32 = mybir.dt.float32
BF16 = mybir.dt.bfloat16
F8 = mybir.dt.float8e4
NPBF = ml_dtypes.bfloat16
NP8 = ml_dtypes.float8_e4m3
DR = mybir.MatmulPerfMode.DoubleRow
WS = 32.0  # host weight pre-scale: keeps fp8 residuals out of subnormals
EXP = mybir.ActivationFunctionType.Exp
ADD = mybir.AluOpType.add
MULT = mybir.AluOpType.mult

_CACHE = {}


def _build_nc():
    nc = bacc.Bacc()
    # x and the (x32-prescaled) QKV weights ship as fp8e4m3 value+residual
    # pairs: projections run as 3 DoubleRow passes (x8.w8 + xr8.w8 + x8.wr8)
    # at 0.75x the bf16 PE cost; the dropped term is quadratic in the
    # quantization error (~0.1%). The 1/32 descale is folded into the
    # cos/sin tables, biases and the V evacuation. All tensors are host
    # pre-arranged so every DMA is 128 contiguous per-partition runs.
    xq = [nc.dram_tensor(f"xq{i}", [128, KCH, 512], F8,
                         kind="ExternalInput") for i in range(4)]
    xqr = [nc.dram_tensor(f"xqr{i}", [128, KCH, 512], F8,
                          kind="ExternalInput") for i in range(4)]
    wq = nc.dram_tensor("wq", [128, KCH, DHL], F8, kind="ExternalInput")
    wqr = nc.dram_tensor("wqr", [128, KCH, DHL], F8, kind="ExternalInput")
    wk = nc.dram_tensor("wk", [128, KCH, DHL], F8, kind="ExternalInput")
    wkr = nc.dram_tensor("wkr", [128, KCH, DHL], F8, kind="ExternalInput")
    wv = nc.dram_tensor("wv", [128, KCH, DHL], F8, kind="ExternalInput")
    wvr = nc.dram_tensor("wvr", [128, KCH, DHL], F8, kind="ExternalInput")
    wo = nc.dram_tensor("wo", [128, QT_TILES, DM], BF16, kind="ExternalInput")
    bq = nc.dram_tensor("bq", [128, QT_TILES], F32, kind="ExternalInput")
    bk = nc.dram_tensor("bk", [128, QT_TILES], F32, kind="ExternalInput")
    # bv is folded into the host-side epilogue: softmax weights sum to 1,
    # so V's bias passes through attention unchanged -> y += bv @ wo
    cosT = nc.dram_tensor("cosT", [128, S], BF16, kind="ExternalInput")
    sinT = nc.dram_tensor("sinT", [128, S], BF16, kind="ExternalInput")
    prot = nc.dram_tensor("prot", [128, 128], BF16, kind="ExternalInput")
    y = nc.dram_tensor("y", [S, DM], BF16, kind="ExternalOutput")

    with TileContext(nc) as tc:
        with (
            tc.tile_pool(name="p0", bufs=1) as p0,
            tc.tile_pool(name="pa_t", bufs=4) as pa_t,
            tc.tile_pool(name="pb_exp", bufs=20) as pb_exp,
            tc.tile_pool(name="pb_n", bufs=3) as pb_n,
            tc.tile_pool(name="pc_y", bufs=4) as pc_y,
            tc.tile_pool(name="ps", bufs=1, space="PSUM") as ps,
        ):
            xq_r = [p0.tile([128, KCH, 512], F8, name=f"xq_r{i}")
                    for i in range(4)]
            xqr_r = [p0.tile([128, KCH, 512], F8, name=f"xqr_r{i}")
                     for i in range(4)]
            wq_r = p0.tile([128, KCH, DHL], F8)
            wqr_r = p0.tile([128, KCH, DHL], F8)
            wk_r = p0.tile([128, KCH, DHL], F8)
            wkr_r = p0.tile([128, KCH, DHL], F8)
            wv_r = p0.tile([128, KCH, DHL], F8)
            wvr_r = p0.tile([128, KCH, DHL], F8)
            wo_r = p0.tile([128, QT_TILES, DM], BF16)
            cos_sb = p0.tile([128, S], BF16)
            sin_sb = p0.tile([128, S], BF16)
            prot_r = p0.tile([128, 128], BF16)
            bq_sb = p0.tile([128, QT_TILES], F32)
            bk_sb = p0.tile([128, QT_TILES], F32)
            ones_row = p0.tile([1, 128], BF16)
            ones_col = p0.tile([128, 1], BF16)
            qrope = p0.tile([128, QT_TILES, S], BF16)
            krope = p0.tile([128, QT_TILES, S], BF16)
            v_r = p0.tile([128, SKT, HL, DH + 1], BF16)
            outT = p0.tile([128, QT_TILES, S], BF16)

            # --- DMAs, in the order phase-B consumption needs them;
            # every transfer is contiguous on both sides ---
            nc.sync.dma_start(wk_r[:], wk[:, :, :])
            nc.sync.dma_start(wkr_r[:], wkr[:, :, :])
            nc.sync.dma_start(bk_sb[:], bk[:, :])
            nc.sync.dma_start(prot_r[:], prot[:, :])
            nc.sync.dma_start(cos_sb[:], cosT[:, :])
            nc.sync.dma_start(sin_sb[:], sinT[:, :])
            nc.sync.dma_start(xq_r[0][:], xq[0][:, :, :])
            nc.sync.dma_start(xqr_r[0][:], xqr[0][:, :, :])
            nc.sync.dma_start(wq_r[:], wq[:, :, :])
            nc.sync.dma_start(wqr_r[:], wqr[:, :, :])
            nc.sync.dma_start(bq_sb[:], bq[:, :])
            nc.sync.dma_start(xq_r[1][:], xq[1][:, :, :])
            nc.sync.dma_start(xqr_r[1][:], xqr[1][:, :, :])
            nc.sync.dma_start(wv_r[:], wv[:, :, :])
            nc.sync.dma_start(wvr_r[:], wvr[:, :, :])
            nc.sync.dma_start(xq_r[2][:], xq[2][:, :, :])
            nc.sync.dma_start(xqr_r[2][:], xqr[2][:, :, :])
            nc.sync.dma_start(xq_r[3][:], xq[3][:, :, :])
            nc.sync.dma_start(xqr_r[3][:], xqr[3][:, :, :])
            nc.sync.dma_start(wo_r[:], wo[:, :, :])

            negc = p0.tile([128, 1], F32)
            nc.vector.memset(negc[:], -4.0)
            nc.vector.memset(ones_row[:], 1.0)
            nc.vector.memset(ones_col[:], 1.0)
            # ones column of V' + preload the exp ACT table while ACT idles
            nc.vector.tensor_copy(
                v_r[:, :, :, DH:DH + 1],
                ones_col[:, None, None, :].broadcast_to([128, SKT, HL, 1]))
            warm = p0.tile([1, 128], F32)
            nc.scalar.activation(warm[:], ones_row[:], EXP, scale=0.125)

            # --- emission helpers ---
            def emit_proj(dest, w_r, wr_r, b_sb, mt, c0, pc=PC):
                """Project + RoPE one pc-col chunk into dest[:, mt, c0:]
                via 3 fp8 DoubleRow passes; the raw psum is WS-scaled,
                which the (pre-divided) cos/sin tables cancel. The prefix
                uses pc=512 to halve the serial rope chain gating the
                first exp."""
                ps_p = ps.tile([128, PC], F32, tag="st", bufs=3)
                ps_p = ps_p[:, 0:pc]
                npass = 3
                for nq in range(pc // 512):
                    qi = (c0 + nq * 512) // 512
                    xv, xr = xq_r[qi], xqr_r[qi]
                    passes = ((xv, w_r), (xr, w_r), (xv, wr_r))
                    for pi, (xa, wa) in enumerate(passes):
                        for j in range(KCH // 2):
                            nc.tensor.matmul(
                                ps_p[:, nq * 512:(nq + 1) * 512],
                                wa[:, 2 * j:2 * j + 2,
                                   mt * 128:(mt + 1) * 128],
                                xa[:, 2 * j:2 * j + 2, :],
                                start=(pi == 0 and j == 0),
                                stop=(pi == npass - 1 and j == KCH // 2 - 1),
                                perf_mode=DR)
                qb_r = pa_t.tile([128, PC], BF16, tag="qb")
                qb_r = qb_r[:, 0:pc]
                nc.vector.tensor_scalar(
                    out=qb_r, in0=ps_p[:],
                    scalar1=b_sb[:, mt:mt + 1], scalar2=None, op0=ADD)
                ps_r = ps.tile([128, PC], F32, tag="st", bufs=3)
                ps_r = ps_r[:, 0:pc]
                for nq in range(pc // 512):
                    nc.tensor.matmul(ps_r[:, nq * 512:(nq + 1) * 512],
                                     prot_r[:, :],
                                     qb_r[:, nq * 512:(nq + 1) * 512],
                                     start=True, stop=True)
                t1 = pa_t.tile([128, PC], F32, tag="t1")
                t1 = t1[:, 0:pc]
                nc.vector.scalar_tensor_tensor(
                    out=t1, in0=ps_p[:], scalar=b_sb[:, mt:mt + 1],
                    in1=cos_sb[:, c0:c0 + pc], op0=ADD, op1=MULT)
                t2 = pa_t.tile([128, PC], F32, tag="t2")
                t2 = t2[:, 0:pc]
                nc.vector.tensor_mul(t2, ps_r, sin_sb[:, c0:c0 + pc])
                nc.vector.tensor_add(dest[:, mt, c0:c0 + pc], t1, t2)

            def emit_v(sk0):
                """V projection for the key-tile pair (sk0, sk0+1); the
                1/WS descale rides the evacuation tensor_scalar."""
                vps = ps.tile([128, PC], F32, tag="st", bufs=3)
                npass = 3
                for i in range(2):
                    sk = sk0 + i
                    qi, skc = divmod(sk * 128, 512)
                    xv, xr = xq_r[qi], xqr_r[qi]
                    passes = ((xv, wv_r), (xr, wv_r), (xv, wvr_r))
                    vp = vps[:, i * 512:i * 512 + DHL]
                    for pi, (xa, wa) in enumerate(passes):
                        for j in range(KCH // 2):
                            nc.tensor.matmul(
                                vp,
                                xa[:, 2 * j:2 * j + 2, skc:skc + 128],
                                wa[:, 2 * j:2 * j + 2, :],
                                start=(pi == 0 and j == 0),
                                stop=(pi == npass - 1 and j == KCH // 2 - 1),
                                perf_mode=DR)
                for i in range(2):
                    sk = sk0 + i
                    nc.vector.tensor_scalar(
                        out=v_r[:, sk, :, 0:DH],
                        in0=vps[:, i * 512:i * 512 + DHL]
                        .rearrange("p (h d) -> p h d", h=HL),
                        scalar1=1.0 / WS, scalar2=None, op0=MULT)

            def emit_phase_c(qt, evac="dve"):
                y_ps = ps.tile([128, DM], F32, tag="st", bufs=3)
                for kc in range(QT_TILES):
                    for c2 in range(DM // 512):
                        nc.tensor.matmul(
                            y_ps[:, c2 * 512:(c2 + 1) * 512],
                            outT[:, kc, qt * 128:(qt + 1) * 128],
                            wo_r[:, kc, c2 * 512:(c2 + 1) * 512],
                            start=(kc == 0), stop=(kc == QT_TILES - 1))
                y_sb = pc_y.tile([128, DM], BF16, tag="ysb")
                # GPSIMD cannot read PSUM on hw -> evacuate via DVE, or
                # via ACT in the tail where the exp stream has ended
                if evac == "act":
                    nc.scalar.activation(y_sb[:], y_ps[:],
                                         mybir.ActivationFunctionType.Copy)
                else:
                    nc.vector.tensor_copy(y_sb[:], y_ps[:])
                nc.sync.dma_start(y[qt * 128:(qt + 1) * 128, :], y_sb[:])

            def eK(mt, c0):
                return lambda: emit_proj(krope, wk_r, wkr_r, bk_sb, mt, c0)

            def eQ(mt, c0):
                return lambda: emit_proj(qrope, wq_r, wqr_r, bq_sb, mt, c0)

            def eV(sk0):
                return lambda: emit_v(sk0)

            def eC(qt):
                return lambda: emit_phase_c(qt)

            # --- minimal phase A prefix: just enough for (qb0, h0, sk0) ---
            emit_proj(krope, wk_r, wkr_r, bk_sb, 0, 0, pc=512)
            emit_proj(qrope, wq_r, wqr_r, bq_sb, 0, 0, pc=512)
            emit_proj(qrope, wq_r, wqr_r, bq_sb, 0, 512, pc=512)
            emit_proj(krope, wk_r, wkr_r, bk_sb, 0, 512, pc=512)
            emit_v(0)

            # remaining phase-A work interleaved into phase B, keyed by
            # (qb, h, sk); each item is emitted right after exp(sk),
            # keeping PE busy without starving ACT
            inter = {
                (0, 0, 0): [eK(0, 1024)],
                (0, 0, 1): [eV(2)],
                (0, 0, 2): [eK(1, 0)],
                (0, 0, 3): [eV(4)],
                (0, 0, 4): [eK(1, 1024)],
                (0, 0, 5): [eV(6)],
                (0, 0, 6): [eV(8)],
                (0, 0, 7): [eV(10)],
                (0, 0, 9): [eV(12)],
                (0, 0, 11): [eV(14)],
                (0, 1, 0): [eQ(1, 0)],
                (0, 1, 4): [eQ(0, 1024)],
                (0, 2, 0): [eQ(1, 1024)],
                # phase C for qb0 (norm(qb0,h3) lands at the end of unit
                # (1,0) under head-lagged PV) spread through qb1's tail
                (1, 1, 1): [eC(0)], (1, 1, 5): [eC(1)],
                (1, 1, 9): [eC(2)], (1, 1, 13): [eC(3)],
                (1, 2, 1): [eC(4)], (1, 2, 5): [eC(5)],
                (1, 2, 9): [eC(6)], (1, 2, 13): [eC(7)],
            }

            def emit_pv(unit, sk):
                qb, h, exs, pv = unit
                for nq in range(QB // 512):
                    nc.tensor.matmul(
                        pv[:, nq * 512:(nq + 1) * 512],
                        v_r[:, sk, h, :],
                        exs[sk][:, nq * 512:(nq + 1) * 512],
                        start=(sk == 0), stop=(sk == SKT - 1))

            def emit_norm(unit, piecewise=False):
                """den row to SBUF (GPSIMD can't read PSUM on hw),
                reciprocal (custom-DVE: SBUF input only), broadcast,
                multiply (hw DVE has no divide ALU op)."""
                qb, h, exs, pv = unit
                q0, mt, prow = qb * QB, h // 2, 64 * (h % 2)
                den = pb_n.tile([1, QB], F32, tag="den")
                nc.vector.tensor_copy(den[0:1, :], pv[DH:DH + 1, :])
                nc.vector.reciprocal_approx_fast(out=den[0:1, :],
                                                 in_=den[0:1, :])
                if not piecewise:
                    rec = pb_n.tile([DH, QB], F32, tag="rec")
                    nc.gpsimd.partition_broadcast(rec[:], den[0:1, :])
                    nc.vector.tensor_mul(
                        outT[prow:prow + DH, mt, q0:q0 + QB],
                        pv[0:DH, :], rec[:])
                else:
                    # norm in 128-col strips (Pool/DVE pipeline), each
                    # strip immediately followed by its output tile
                    for j in range(QB // 128):
                        c = j * 128
                        rec = pb_n.tile([DH, 128], F32, tag="recp",
                                        bufs=4)
                        nc.gpsimd.partition_broadcast(
                            rec[:], den[0:1, c:c + 128])
                        nc.vector.tensor_mul(
                            outT[prow:prow + DH, mt, q0 + c:q0 + c + 128],
                            pv[0:DH, c:c + 128], rec[:])
                        if j >= 1:
                            emit_phase_c(8 * qb + j - 1,
                                         evac="act" if j % 2 else "dve")
                    emit_phase_c(8 * qb + QB // 128 - 1, evac="act")

            # PV for head unit N runs one unit late (during unit N+1's
            # exp stream): PE never waits on ACT, and unit 0's window is
            # free for the projection backlog
            prev = None
            for qb in range(NQB):
                q0 = qb * QB
                for h in range(HL):
                    final = qb == NQB - 1 and h == HL - 1
                    mt, prow = h // 2, 64 * (h % 2)
                    kt_h = krope[prow:prow + DH, mt, :]
                    qt_h = qrope[prow:prow + DH, mt, :]
                    exs = [None] * SKT
                    pv_t = ps.tile([DH + 1, QB], F32, tag="pv", bufs=1)
                    cur = (qb, h, exs, pv_t)
                    for sk in range(SKT):
                        st_ps = ps.tile([128, QB], F32, tag="st", bufs=3)
                        for nq in range(QB // 512):
                            nc.tensor.matmul(
                                st_ps[:, nq * 512:(nq + 1) * 512],
                                kt_h[:, sk * 128:(sk + 1) * 128],
                                qt_h[:, q0 + nq * 512:q0 + (nq + 1) * 512],
                                start=True, stop=True)
                        ex = pb_exp.tile([128, QB], BF16, tag="ex")
                        nc.scalar.activation(ex[:], st_ps[:], EXP,
                                             scale=0.125, bias=negc[:])
                        exs[sk] = ex
                        for fn in inter.get((qb, h, sk), ()):
                            fn()
                        if prev is not None:
                            if not final:
                                emit_pv(prev, sk)
                            else:
                                # last unit: drain the previous head's PVs
                                # densely up front (their exps are long
                                # done), norm it mid-unit, then run this
                                # unit's own PVs at a short lag so almost
                                # nothing is left after the final exp
                                if sk < 8:
                                    emit_pv(prev, 2 * sk)
                                    emit_pv(prev, 2 * sk + 1)
                                    if sk == 7:
                                        emit_norm(prev)
                                elif sk >= 10:
                                    emit_pv(cur, sk - 10)
                    if prev is not None and not final:
                        emit_norm(prev)
                    prev = cur
            # short drain: only the last 10 PVs remain, then piecewise
            # norm + output tiles
            for sk in range(SKT - 10, SKT):
                emit_pv(prev, sk)
            emit_norm(prev, piecewise=True)

    nc.finalize()
    return nc


def _rope_tables():
    inv_freq = 1.0 / (10000.0 ** (np.arange(0, DH, 2, dtype=np.float32) / DH))
    ang = np.arange(S, dtype=np.float32)[:, None] * inv_freq[None, :]
    sin = np.concatenate([np.sin(ang), np.sin(ang)], axis=-1)  # [S, DH]
    cos = np.concatenate([np.cos(ang), np.cos(ang)], axis=-1)
    sinT = (np.ascontiguousarray(np.vstack([sin.T, sin.T]))
            / WS).astype(NPBF)
    cosT = (np.ascontiguousarray(np.vstack([cos.T, cos.T]))
            / WS).astype(NPBF)
    return sinT, cosT  # [128, S] bf16, pre-divided by the weight scale


def _rot_matrix():
    half = DH // 2
    m64 = np.zeros((DH, DH), dtype=np.float32)
    for d in range(half):
        m64[d + half, d] = -1.0       # rot[d] = -q[d+half]
    for d in range(half, DH):
        m64[d - half, d] = 1.0        # rot[d] = q[d-half]
    m = np.zeros((128, 128), dtype=np.float32)
    m[0:DH, 0:DH] = m64
    m[DH:, DH:] = m64
    return m.astype(NPBF)


def _make_runner(nc):
    """Build a cached jitted SPMD executor (mirrors the multi-core tail of
    concourse.bass2jax.run_bass_via_pjrt so repeat calls skip recompiles)."""
    import jax
    import numpy as _np
    from jax.sharding import Mesh, PartitionSpec
    from jax.experimental.shard_map import shard_map
    from concourse import bass2jax, mybir as _mybir

    bass2jax.install_neuronx_cc_hook()

    partition_name = (
        nc.partition_id_tensor.name if nc.partition_id_tensor else None)
    in_names, out_names, out_avals, zero_shapes = [], [], [], []
    for alloc in nc.m.functions[0].allocations:
        if not isinstance(alloc, _mybir.MemoryLocationSet):
            continue
        name = alloc.memorylocations[0].name
        if alloc.kind == "ExternalInput":
            if name != partition_name:
                in_names.append(name)
        elif alloc.kind == "ExternalOutput":
            out_names.append(name)
            shape = tuple(alloc.tensor_shape)
            dtype = _mybir.dt.np(alloc.dtype)
            out_avals.append(jax.core.ShapedArray(shape, dtype))
            zero_shapes.append((shape, dtype))
    n_params = len(in_names)
    all_names = in_names + out_names
    if partition_name is not None:
        all_names = all_names + [partition_name]

    def _body(*args):
        operands = list(args)
        if partition_name is not None:
            operands.append(bass2jax.partition_id_tensor())
        outs = bass2jax._bass_exec_p.bind(
            *operands,
            out_avals=tuple(out_avals),
            in_names=tuple(all_names),
            out_names=tuple(out_names),
            lowering_input_output_aliases=(),
            sim_require_finite=True,
            sim_require_nnan=True,
            nc=nc,
        )
        return tuple(outs)

    devices = jax.devices()[:NCORES]
    mesh = Mesh(_np.asarray(devices), ("core",))
    n_outs = len(out_names)
    sharded = jax.jit(
        shard_map(
            _body, mesh=mesh,
            in_specs=(PartitionSpec("core"),) * (n_params + n_outs),
            out_specs=(PartitionSpec("core"),) * n_outs,
            check_rep=False,
        ),
        donate_argnums=tuple(range(n_params, n_params + n_outs)),
        keep_unused=True,
    )

    def run(in_maps):
        concat_in = [
            _np.concatenate([_np.asarray(m[name]) for m in in_maps], axis=0)
            for name in in_names
        ]
        concat_zeros = [
            _np.zeros((NCORES * s[0], *s[1:]), dt) for (s, dt) in zero_shapes
        ]
        out_arrs = sharded(*concat_in, *concat_zeros)
        return [
            {
                name: _np.asarray(out_arrs[i]).reshape(
                    NCORES, *out_avals[i].shape)[c]
                for i, name in enumerate(out_names)
            }
            for c in range(NCORES)
        ]

    return run


def _get_runner():
    if "runner" not in _CACHE:
        nc = _build_nc()
        _CACHE["nc"] = nc
        _CACHE["runner"] = _make_runner(nc)
    return _CACHE["runner"]


def make_in_maps(x, wq, bq, wk, bk, wv, bv, wo, bo):
    """Build the 8 per-core input dicts from full inputs (bf16 on host)."""
    x = np.asarray(x, dtype=np.float32)
    if "tables" not in _CACHE:
        _CACHE["tables"] = _rope_tables()
        _CACHE["prot"] = _rot_matrix()
    sinT, cosT = _CACHE["tables"]
    prot = _CACHE["prot"]

    def split8(a):
        """fp8e4m3 value + quantized residual (error ~ (0.04)^2 |a|)."""
        v8 = a.astype(NP8)
        r8 = (a - v8.astype(np.float32)).astype(NP8)
        return v8, r8

    def sb_layout(a):  # [DM, n] -> [128, KCH, n]
        return np.ascontiguousarray(
            a.reshape(KCH, 128, -1).transpose(1, 0, 2))

    xb = []
    for b in range(B):
        xt = x[b].T
        xb.append([split8(sb_layout(
            np.ascontiguousarray(xt[:, i * 512:(i + 1) * 512])))
            for i in range(4)])
    in_maps = []
    for c in range(NCORES):
        b, hg = divmod(c, HL)
        sl = slice(hg * DHL, (hg + 1) * DHL)
        wq8, wqr8 = split8(sb_layout(
            np.asarray(wq, np.float32)[:, sl] * WS))
        wk8, wkr8 = split8(sb_layout(
            np.asarray(wk, np.float32)[:, sl] * WS))
        wv8, wvr8 = split8(sb_layout(
            np.asarray(wv, np.float32)[:, sl] * WS))
        xmap = {}
        for i in range(4):
            xmap[f"xq{i}"] = xb[b][i][0]
            xmap[f"xqr{i}"] = xb[b][i][1]
        in_maps.append({
            **xmap,
            "wq": wq8, "wqr": wqr8,
            "wk": wk8, "wkr": wkr8,
            "wv": wv8, "wvr": wvr8,
            "wo": np.ascontiguousarray(
                np.asarray(wo, np.float32)[sl, :].reshape(QT_TILES, 128, DM)
                .transpose(1, 0, 2)).astype(NPBF),
            "bq": np.ascontiguousarray(
                np.asarray(bq, np.float32)[sl].reshape(QT_TILES, 128).T
                * WS),
            "bk": np.ascontiguousarray(
                np.asarray(bk, np.float32)[sl].reshape(QT_TILES, 128).T
                * WS),
            "cosT": cosT,
            "sinT": sinT,
            "prot": prot,
        })
    return in_maps


def kernel(x, wq, bq, wk, bk, wv, bv, wo, bo):
    runner = _get_runner()
    in_maps = make_in_maps(x, wq, bq, wk, bk, wv, bv, wo, bo)
    results = runner(in_maps)
    bo = np.asarray(bo, dtype=np.float32)
    # bv passes through the softmax average unchanged -> fold into epilogue
    bias = (np.asarray(bv, np.float32) @ np.asarray(wo, np.float32)) + bo
    out = np.empty((B, S, DM), dtype=np.float32)
    for b in range(B):
        acc = results[b * HL + 0]["y"].astype(np.float32)
        for hg in range(1, HL):
            acc += results[b * HL + hg]["y"].astype(np.float32)
        out[b] = acc + bias[None, :]
    return out
